# revision 32
# baseline (speedup 1.0000x reference)
"""Multi-head attention with q/v LoRA on 8 trn2 NeuronCores (v3).

Reference computation (B=2, N=2048, C=1024, H=16, HD=64, R=16):
    qkv = x @ w_qkv + b_qkv                -> split per-head q, k, v
    q  += ((q @ a_q) @ b_q) * 2.0          (per head; same for v)
    out = softmax(q k^T / 8) v             (full N x N scores)
    y   = out @ w_proj + b_proj

Sharding: tensor-parallel over heads -- each of the 8 cores owns 2 heads
for both batches; attention output resharded over tokens with one
AllToAll per (batch, query-half) so each core computes final proj rows
for its 256 tokens per batch against the full w_proj.

v3 structure (vs the 338us per-head-unit v2):
  * superunits process BOTH heads of one (batch, q-half).  The scores
    PSUM tiles pack [h0 | h1] per query-half (s0 = qc0, s1 = qc1), so
    the two heads' score matmuls are adjacent (0,0)/(64,0) row-tiles
    that run CONCURRENTLY on the PE array -- halving scores PE time.
  * exp runs on [128,1024] tiles (both heads per qc-half); the kt loop
    is ACT-bound (~2.2us/kt) while attention PE needs ~1.3us/kt, and
    the slack is drained from two thunk queues:
      CRIT: previous SU's deferred qc1 PV matmuls (reading exp tiles
            retained in SBUF), its o-bank evac, and its normalize/
            stage/a2a-fire -- these gate PSUM bank rotation and the
            collectives, so they pop first.
      BULK: qkv(b1) m-units and proj tiles.
  * pv0 emission is back-logged while CRIT is nonempty so a PSUM WAR
    on the o banks can never freeze the in-order PE FIFO.
  * one AllToAll per (b, qh), fired from the deferred finish ~mid next
    SU; proj per (b, qh) runs as BULK thunks two SUs later; the last
    SU runs its qc1/finish inline so its a2a fires immediately.
"""

import sys

sys.path.insert(0, "/opt/trn_rl_repo")
sys.path.insert(0, "/root/.axon_site")

import numpy as np
import ml_dtypes

import concourse.bass as bass
import concourse.mybir as mybir
import concourse.tile as tile
from concourse.bass_utils import run_bass_kernel_spmd

f32 = mybir.dt.float32
f32r = mybir.dt.float32r
bf16 = mybir.dt.bfloat16
fp8 = mybir.dt.float8e4
DR = mybir.MatmulPerfMode.DoubleRow
AF = mybir.ActivationFunctionType

B, N, C = 2, 2048, 1024
H, HD, R = 16, 64, 16
LORA_SCALE = 32.0 / R
ATTN_SCALE = HD ** -0.5
NCORES = 8
HPC = H // NCORES          # heads per core = 2
PC = HPC * HD              # partition columns per core = 128
ROWS = B * N               # 4096 tokens
RC = 256                   # row-chunk size for qkv production
TPC = N // NCORES          # tokens per core per batch = 256


def _legalize_waits(nc, max_waits=1):
    """This walrus build accepts at most one sync-wait per instruction;
    Tile attaches several.  Move surplus waits onto same-engine NoOps
    inserted immediately before the instruction (identical semantics)."""
    counter = 0
    for fn in nc.m.functions:
        for bb in fn.blocks:
            insts = bb.instructions
            out = []
            changed = False
            for inst in insts:
                si = inst.sync_info
                if si is not None and si.on_wait and len(si.on_wait) > max_waits:
                    waits = list(si.on_wait)
                    for w in waits[:-max_waits]:
                        counter += 1
                        nop = mybir.InstNoOp(
                            name=f"I-wfix-{counter}",
                            engine=inst.engine,
                            sync_info=mybir.SyncInfo(on_wait=[w], on_update=[]),
                        )
                        nc.register_instruction(nop)
                        out.append(nop)
                    si.on_wait.clear()
                    si.on_wait.extend(waits[-max_waits:])
                    changed = True
                out.append(inst)
            if changed:
                insts[:] = out


def build_nc():
    nc = bass.Bass(num_devices=NCORES)

    xt_d = nc.dram_tensor("xt", [C, ROWS], bf16, kind="ExternalInput")
    wq_d = nc.dram_tensor("wq", [128, 1024], bf16, kind="ExternalInput")
    wk_d = nc.dram_tensor("wk", [128, 1024], bf16, kind="ExternalInput")
    wv_d = nc.dram_tensor("wv", [128, 1024], bf16, kind="ExternalInput")
    bq_d = nc.dram_tensor("bq", [128, 1], f32, kind="ExternalInput")
    bk_d = nc.dram_tensor("bk", [128, 1], f32, kind="ExternalInput")
    bv_d = nc.dram_tensor("bv", [128, 1], f32, kind="ExternalInput")
    wp_d = nc.dram_tensor("wp", [128, 8 * 1024], bf16, kind="ExternalInput")
    bp_d = nc.dram_tensor("bp", [128, 8], f32, kind="ExternalInput")
    out_d = nc.dram_tensor("out", [B, C, TPC], f32, kind="ExternalOutput")

    with nc.allow_low_precision(
        reason="bf16 matmul operands are intended; PSUM accumulation stays fp32"
    ), tile.TileContext(nc) as tc:
        with (
            tc.tile_pool(name="persist", bufs=1) as persist,
            tc.tile_pool(name="const", bufs=1) as const,
            tc.tile_pool(name="dram", bufs=1, space="DRAM") as dram,
            tc.tile_pool(name="xio", bufs=6) as xio_p,
            tc.tile_pool(name="work", bufs=2) as work_p,
            tc.tile_pool(name="ps", bufs=1, space="PSUM") as ps,
        ):
            qT = persist.tile([128, ROWS], bf16, tag="qT", name="qT")
            kT = persist.tile([128, ROWS], bf16, tag="kT", name="kT")
            # v^T per head, padded to 80 rows: row 64 = ones (the PV sums
            # row), rows 65-79 = filler so the XBAR transpose DMA writes a
            # CONTIGUOUS [128, 16*80] v_aug (non-contiguous transpose
            # destinations are broken on hardware)
            vT_h = [
                persist.tile([80, ROWS], bf16, tag=f"vT{h}", name=f"vT{h}")
                for h in range(2)
            ]
            for h in range(2):
                nc.gpsimd.memset(vT_h[h][64:80, :], 1.0)

            xstg = {}

            def xchunk_dma(b, rci):
                r0 = b * N + rci * RC
                t = xio_p.tile([128, 8 * RC], bf16, tag="xstg", name=f"xs{b}{rci}")
                nc.sync.dma_start(
                    out=t[:].rearrange("p (a r) -> p a r", a=8),
                    in_=xt_d[:, r0 : r0 + RC].rearrange("(a p) r -> p a r", p=128),
                )
                xstg[(b, rci)] = t

            xchunk_dma(0, 0)
            xchunk_dma(0, 1)

            def loaded(name, dram_t, shape, dt):
                t = const.tile(list(shape), dt, tag=name, name=name)
                nc.sync.dma_start(out=t[:], in_=dram_t[:])
                return t

            w_t = [
                loaded("wq_t", wq_d, (128, 1024), bf16),
                loaded("wk_t", wk_d, (128, 1024), bf16),
                loaded("wv_t", wv_d, (128, 1024), bf16),
            ]
            bias_t = [
                loaded("bq", bq_d, (128, 1), f32),
                loaded("bk", bk_d, (128, 1), f32),
                loaded("bv", bv_d, (128, 1), f32),
            ]
            bp_t = loaded("bp", bp_d, (128, 8), f32)

            for rci in range(2, 8):
                xchunk_dma(0, rci)
            for rci in range(8):
                xchunk_dma(1, rci)

            ones_s = const.tile([1, 64], f32, tag="ones_s", name="ones_s")
            nc.gpsimd.memset(ones_s[:], 1.0)
            ones_row = const.tile([1, 64], f32r, tag="ones_r", name="ones_r")
            nc.vector.tensor_copy(ones_row[:], ones_s[:])

            wp_t = const.tile([128, 8 * 1024], bf16, tag="wp_t", name="wp_t")

            # ---- pending PE-work queues ------------------------------------
            CRIT = []   # (cost_ns, thunk): deferred PV-qc1 / evac / finish
            BULK = []   # (cost_ns, kind, thunk): qkv m-units, proj tiles

            def pop_thunks(budget=900):
                spent = 0
                while CRIT:
                    cost, th = CRIT[0]
                    if spent and spent + cost > budget:
                        return
                    CRIT.pop(0)
                    th()
                    spent += cost
                while BULK:
                    cost, _, th = BULK[0]
                    if spent and spent + cost > budget:
                        return
                    BULK.pop(0)
                    th()
                    spent += cost

            def drain_crit():
                while CRIT:
                    _, th = CRIT.pop(0)
                    th()

            def drain_bulk(kind=None):
                rest = []
                while BULK:
                    cost, k, th = BULK.pop(0)
                    if kind is None or k == kind:
                        th()
                    else:
                        rest.append((cost, k, th))
                BULK.extend(rest)

            def force_chunk(b, rci):
                # emission-order safety: attention matmuls may only be
                # emitted AFTER the qkv m-units that write the regions they
                # read (Tile tracks dependencies in emission order)
                while qkv_emitted.get((b, rci), 0) < 3:
                    assert BULK, f"qkv chunk ({b},{rci}) unsatisfiable"
                    _, _, th = BULK.pop(0)
                    th()

            # ---- qkv m-units (one [128,256] output tile: 8 MMs + bias) ----
            qkv_emitted = {}

            def emit_qkv_munit(b, rci, m):
                qkv_emitted[(b, rci)] = qkv_emitted.get((b, rci), 0) + 1
                r0 = b * N + rci * RC
                xs = xstg[(b, rci)]
                acc = ps.tile([128, RC], f32, tag="qacc", bufs=1, name=f"qa{b}{rci}{m}")
                for ci in range(8):
                    nc.tensor.matmul(
                        acc[:],
                        w_t[m][:, ci * 128 : (ci + 1) * 128],
                        xs[:, ci * RC : (ci + 1) * RC],
                        start=(ci == 0),
                        stop=(ci == 7),
                    )
                if m < 2:
                    dst = (qT, kT)[m]
                    nc.vector.tensor_scalar_add(
                        dst[:, r0 : r0 + RC], acc[:], bias_t[m][:]
                    )
                else:
                    # v splits into the two per-head padded tiles
                    for h in range(2):
                        nc.vector.tensor_scalar_add(
                            vT_h[h][0:64, r0 : r0 + RC],
                            acc[h * 64 : (h + 1) * 64, :],
                            bias_t[m][h * 64 : (h + 1) * 64, :],
                        )

            def queue_qkv_chunk(b, rci):
                for m in range(3):
                    BULK.append(
                        (870, "qkv", lambda b=b, rci=rci, m=m: emit_qkv_munit(b, rci, m))
                    )

            # ---- v_aug: keys-major [128, 16*80] = per-kt [V_h | ones | pad],
            # built per key-half by an XBAR transposing DMA from the padded
            # vT_h tile (no PE, no PSUM), then DVE-cast to fp8 for the
            # DoubleRow PV matmuls
            def make_vaug(b, hl):
                return work_p.tile([128, 16 * 80], bf16, tag=f"vgb{hl}", bufs=2,
                                   name=f"vb{b}{hl}")

            def emit_vaug_half(b, hl, half, va):
                boff = b * N + half * 1024
                co = half * 8 * 80
                nc.sync.dma_start_transpose(
                    out=va[:, co : co + 640].rearrange("p (k c) -> p k c", c=80),
                    in_=vT_h[hl][:, boff : boff + 1024],
                )

            def act_recip_row(r_out, s_in, lg):
                """1/s = exp(-ln(s)) on ACT: Ln and Exp share the softmax
                exp's table set, so no table reload."""
                nc.scalar.activation(lg[:], s_in, AF.Ln)
                nc.scalar.activation(r_out, lg[:], AF.Exp, scale=-1.0)

            # ---- a2a plumbing ---------------------------------------------
            ai = {}
            for b in range(B):
                for qh in range(2):
                    ai[(b, qh)] = dram.tile([8, 128, 128], bf16,
                                            tag=f"ai{b}{qh}", name=f"ai{b}{qh}")

            recv_tiles = {}

            def get_recv(b, qh):
                # one tile per (b, qh) so proj thunks depend ONLY on their
                # own a2a's landing (a shared tile made proj(0,0) wait on
                # recv(0,1) via coarse dependency tracking -- 25us stall)
                if (b, qh) not in recv_tiles:
                    recv_tiles[(b, qh)] = work_p.tile(
                        [128, 8 * 128], bf16, tag=f"rcr{b}{qh}", bufs=1, name=f"rr{b}{qh}"
                    )
                return recv_tiles[(b, qh)]

            def fire_a2a(b, qh, split_recv=False):
                a2a_out = dram.tile([8, 128, 128], bf16, tag=f"ao{b}{qh}", name=f"ao{b}{qh}")
                nc.gpsimd.collective_compute(
                    "AllToAll",
                    mybir.AluOpType.bypass,
                    replica_groups=[list(range(NCORES))],
                    ins=[ai[(b, qh)][:].opt()],
                    outs=[a2a_out[:].opt()],
                )
                recv_r = get_recv(b, qh)
                if split_recv:
                    for kc in range(8):
                        nc.sync.dma_start(
                            out=recv_r[:, kc * 128 : (kc + 1) * 128],
                            in_=a2a_out[kc, :, :],
                        )
                else:
                    nc.sync.dma_start(
                        out=recv_r[:].rearrange("p (c t) -> p c t", t=128),
                        in_=a2a_out[:].rearrange("c p t -> p c t"),
                    )

            # ---- superunit: both heads of one (batch, q-half) --------------
            def emit_su(b, qh, v_augs, pair_gate=None, inline_tail=False):
                """Scores (row-tiled concurrent pairs) -> exp -> bf16 PV.

                s0 = [h0qc0 | h1qc0], s1 = [h0qc1 | h1qc1]; one exp per
                (kt, qc-half).  PV for qc0 accumulates in-loop into
                [65,512] banks (backlogged while CRIT is nonempty or a
                v_aug half is not yet built); qc1 PV is deferred into CRIT
                with the evac and the normalize/stage/a2a-fire finish.
                The reciprocals run on the DVE (off the exp-critical ACT
                stream) except for the last SU, where ACT's lower latency
                matters for the tail.
                """
                if pair_gate is None:
                    pair_gate = [16]
                boff = b * N
                qoff = boff + qh * 1024
                for rci in range(qh * 4, qh * 4 + 4):
                    force_chunk(b, rci)
                o_qc0 = [
                    ps.tile([65, 512], f32, tag=f"oh{h}", bufs=1, name=f"o{b}{qh}{h}0")
                    for h in range(2)
                ]
                p0_tiles = {}
                p1_tiles = {}
                pv0_backlog = []

                def emit_pv0(kt):
                    p0 = p0_tiles.pop(kt)
                    for h in range(2):
                        nc.tensor.matmul(
                            o_qc0[h][:],
                            v_augs[h][:, kt * 80 : kt * 80 + 65],
                            p0[:, h * 512 : (h + 1) * 512],
                            start=(kt == 0),
                            stop=(kt == 15),
                        )

                def flush_pv0(upto, maxn=3):
                    if CRIT:
                        return
                    n = 0
                    while (pv0_backlog and pv0_backlog[0] <= upto
                           and pv0_backlog[0] // 2 < pair_gate[0] and n < maxn):
                        emit_pv0(pv0_backlog.pop(0))
                        n += 1

                for kt in range(16):
                    force_chunk(b, kt // 2)
                    ko = boff + kt * 128
                    s0 = ps.tile([128, 1024], f32, tag="s0", bufs=1, name=f"s0_{b}{qh}{kt}")
                    s1 = ps.tile([128, 1024], f32, tag="s1", bufs=1, name=f"s1_{b}{qh}{kt}")
                    # concurrent row-tiled pairs: h0 rows 0-63, h1 rows 64-127
                    for h, cs in ((0, slice(0, 512)), (1, slice(512, 1024))):
                        hs = slice(h * HD, (h + 1) * HD)
                        nc.tensor.matmul(
                            s0[:, cs],
                            kT[hs, ko : ko + 128],
                            qT[hs, qoff : qoff + 512],
                            start=True, stop=True,
                        )
                    for h, cs in ((0, slice(0, 512)), (1, slice(512, 1024))):
                        hs = slice(h * HD, (h + 1) * HD)
                        nc.tensor.matmul(
                            s1[:, cs],
                            kT[hs, ko : ko + 128],
                            qT[hs, qoff + 512 : qoff + 1024],
                            start=True, stop=True,
                        )
                    p0 = work_p.tile([128, 1024], bf16, tag="p0", bufs=16, name=f"p0_{qh}{kt}")
                    p1 = work_p.tile([128, 1024], bf16, tag="p1", bufs=17, name=f"p1_{qh}{kt}")
                    nc.scalar.activation(p0[:], s0[:], AF.Exp, scale=ATTN_SCALE)
                    nc.scalar.activation(p1[:], s1[:], AF.Exp, scale=ATTN_SCALE)
                    p0_tiles[kt] = p0
                    p1_tiles[kt] = p1
                    pv0_backlog.append(kt)

                    pop_thunks(1400 if inline_tail else 900)
                    flush_pv0(kt - 1)

                drain_crit()
                while pv0_backlog:
                    if pv0_backlog[0] // 2 >= pair_gate[0] and BULK:
                        # pv0 gated on a v_aug half not yet emitted: force
                        # BULK forward until the gate-opening thunk runs
                        _, _, th = BULK.pop(0)
                        th()
                        continue
                    emit_pv0(pv0_backlog.pop(0))

                # qc0 evac: frees the o banks for the deferred qc1 pass
                nsth = [
                    work_p.tile([65, 1024], f32r, tag=f"nsth{h}", bufs=1, name=f"nh{b}{qh}{h}")
                    for h in range(2)
                ]
                for h in range(2):
                    nc.vector.tensor_copy(nsth[h][:, 0:512], o_qc0[h][:])

                # qc1 accumulators allocated EAGERLY here so the oh-tag
                # rotation order is (this SU qc0) -> (this SU qc1) ->
                # (next SU qc0); a lazy allocation inside the thunk would
                # invert the last two and deadlock the PE FIFO.
                o_qc1 = [
                    ps.tile([65, 512], f32, tag=f"oh{h}", bufs=1, name=f"o{b}{qh}{h}1")
                    for h in range(2)
                ]

                def pv1_thunk(kt):
                    p1 = p1_tiles.pop(kt)
                    for h in range(2):
                        nc.tensor.matmul(
                            o_qc1[h][:],
                            v_augs[h][:, kt * 80 : kt * 80 + 65],
                            p1[:, h * 512 : (h + 1) * 512],
                            start=(kt == 0),
                            stop=(kt == 15),
                        )

                def evac_qc1():
                    for h in range(2):
                        nc.vector.tensor_copy(nsth[h][:, 512:1024], o_qc1[h][:])

                def finish(split_recv=False):
                    nstb = work_p.tile([128, 1024], bf16, tag="nstb", bufs=2, name=f"nb{b}{qh}")
                    for h in range(2):
                        rr = work_p.tile([1, 1024], f32r, tag="r2", bufs=2, name=f"r{b}{qh}{h}")
                        lg = work_p.tile([1, 1024], f32, tag="lg", bufs=2,
                                         name=f"lg{b}{qh}{h}")
                        act_recip_row(rr[:], nsth[h][64:65, :], lg)
                        for qc in range(2):
                            bc = ps.tile([64, 512], f32, tag="bc", bufs=1, name=f"bc{h}{qc}")
                            nc.tensor.matmul(
                                bc[:],
                                ones_row[:],
                                rr[:, qc * 512 : (qc + 1) * 512],
                                start=True, stop=True,
                            )
                            nc.vector.tensor_mul(
                                nstb[h * 64 : (h + 1) * 64, qc * 512 : (qc + 1) * 512],
                                nsth[h][0:64, qc * 512 : (qc + 1) * 512],
                                bc[:],
                            )
                    # stage: dst core c gets its 128-token slice from each head
                    nc.sync.dma_start(
                        out=ai[(b, qh)][:].rearrange("c p t -> p c t"),
                        in_=nstb[:].rearrange("p (c t) -> p c t", c=8),
                    )
                    fire_a2a(b, qh, split_recv=split_recv)

                if inline_tail:
                    for kt in range(16):
                        pv1_thunk(kt)
                    evac_qc1()
                    finish(split_recv=True)
                else:
                    for kt in range(16):
                        CRIT.append((440, lambda kt=kt: pv1_thunk(kt)))
                    CRIT.append((700, evac_qc1))
                    CRIT.append((3000, finish))

            # ---- proj: one [128, 128] output tile per (b, qh, mt) ----------
            y_sb = {}

            def get_ysb(b, qh):
                if (b, qh) not in y_sb:
                    y_sb[(b, qh)] = work_p.tile(
                        [128, 8 * 128], f32, tag="ysb", bufs=4, name=f"y{b}{qh}"
                    )
                return y_sb[(b, qh)]

            def proj_mt(b, qh, mt):
                recv_r = get_recv(b, qh)
                ys = get_ysb(b, qh)
                y_ps = ps.tile([128, 128], f32, tag="qacc", bufs=1, name=f"yp{b}{qh}{mt}")
                for kc in range(8):
                    nc.tensor.matmul(
                        y_ps[:],
                        wp_t[:, kc * 1024 + mt * 128 : kc * 1024 + (mt + 1) * 128],
                        recv_r[:, kc * 128 : (kc + 1) * 128],
                        start=(kc == 0),
                        stop=(kc == 7),
                    )
                nc.vector.tensor_scalar_add(
                    ys[:, mt * 128 : (mt + 1) * 128], y_ps[:], bp_t[:, mt : mt + 1]
                )

            def queue_proj(b, qh):
                for mt in range(8):
                    BULK.append((870, "proj", lambda b=b, qh=qh, mt=mt: proj_mt(b, qh, mt)))

            def out_dma(b, qh):
                ys = get_ysb(b, qh)
                nc.sync.dma_start(
                    out=out_d[b].rearrange("(a p) t -> p a t", p=128)[
                        :, :, qh * 128 : (qh + 1) * 128
                    ],
                    in_=ys[:].rearrange("p (a t) -> p a t", a=8),
                )

            # ---- emission schedule ----------------------------------------
            # qkv(b0) first half up-front; chunks 4-7 + the second v_aug
            # halves drain inside SU(0,0) (its kt 8-15 consume them just in
            # time), so attention starts ~10us earlier
            for rci in range(4):
                for m in range(3):
                    emit_qkv_munit(0, rci, m)
            nc.sync.dma_start(out=wp_t[:], in_=wp_d[:])

            va0 = (make_vaug(0, 0), make_vaug(0, 1))
            for hl in range(2):
                emit_vaug_half(0, hl, 0, va0[hl])
            gate00 = [4]
            for rci in range(4, 8):
                queue_qkv_chunk(0, rci)

            def open_gate00():
                for hl in range(2):
                    emit_vaug_half(0, hl, 1, va0[hl])
                gate00[0] = 8

            BULK.append((100, "qkv", open_gate00))
            for rci in range(8):
                queue_qkv_chunk(1, rci)

            emit_su(0, 0, va0, pair_gate=gate00)
            emit_su(0, 1, va0)
            # vaug(1) reads all of vT(b1): force remaining qkv thunks out
            drain_bulk("qkv")
            va1 = (make_vaug(1, 0), make_vaug(1, 1))
            for hl in range(2):
                emit_vaug_half(1, hl, 0, va1[hl])
                emit_vaug_half(1, hl, 1, va1[hl])
            emit_su(1, 0, va1)
            # proj(0,*) data landed during SU(1,0); its tiles fill SU(1,1)'s
            # PE slack (keeping HAM warm into the tail).  proj(1,0) stays
            # for the tail to cover the last a2a's latency.
            queue_proj(0, 0)
            queue_proj(0, 1)
            emit_su(1, 1, va1, inline_tail=True)
            queue_proj(1, 0)
            drain_bulk()
            out_dma(0, 0)
            out_dma(0, 1)
            out_dma(1, 0)
            for mt in range(8):
                proj_mt(1, 1, mt)
            out_dma(1, 1)

    _legalize_waits(nc)
    return nc


_NC_CACHE = None


def _get_nc():
    global _NC_CACHE
    if _NC_CACHE is None:
        _NC_CACHE = build_nc()
    return _NC_CACHE


def _make_in_maps(inputs):
    bf = ml_dtypes.bfloat16
    x = np.ascontiguousarray(np.asarray(inputs["x"], dtype=np.float32)).reshape(ROWS, C)
    xt = np.ascontiguousarray(x.T).astype(bf)   # [C, ROWS] bf16
    w_qkv = np.asarray(inputs["w_qkv"], dtype=np.float64)
    b_qkv = np.asarray(inputs["b_qkv"], dtype=np.float64)
    a_q = np.asarray(inputs["a_q"], dtype=np.float64)
    b_q = np.asarray(inputs["b_q"], dtype=np.float64)
    a_v = np.asarray(inputs["a_v"], dtype=np.float64)
    b_v = np.asarray(inputs["b_v"], dtype=np.float64)
    w_proj = np.asarray(inputs["w_proj"], dtype=np.float32)
    b_proj = np.asarray(inputs["b_proj"], dtype=np.float32)

    # fold per-head LoRA into the q/v projections:  q' = q (I + A B s)
    m_q = np.eye(HD) + a_q @ b_q * LORA_SCALE          # [64, 64]
    m_v = np.eye(HD) + a_v @ b_v * LORA_SCALE
    wq = np.ascontiguousarray(
        (w_qkv[:, 0 * C : 1 * C].reshape(C, H, HD) @ m_q).reshape(C, C)
    )
    wk = w_qkv[:, 1 * C : 2 * C]
    wv = np.ascontiguousarray(
        (w_qkv[:, 2 * C : 3 * C].reshape(C, H, HD) @ m_v).reshape(C, C)
    )
    bq = (b_qkv[0 * C : 1 * C].reshape(H, HD) @ m_q).reshape(C).astype(np.float32)
    bk = b_qkv[1 * C : 2 * C].astype(np.float32)
    bv = (b_qkv[2 * C : 3 * C].reshape(H, HD) @ m_v).reshape(C).astype(np.float32)

    def warr(w):                              # [1024, n] -> [128, 8*n] chunk-major
        n = w.shape[1]
        return np.ascontiguousarray(
            w.reshape(8, 128, n).transpose(1, 0, 2).reshape(128, 8 * n)
        ).astype(bf)

    wp_full = warr(w_proj)                    # [128, 8*1024] bf16
    bp = np.ascontiguousarray(b_proj.reshape(8, 128).T)

    in_maps = []
    for c in range(NCORES):
        sl = slice(c * PC, (c + 1) * PC)
        in_maps.append(
            {
                "xt": xt,
                "wq": warr(wq[:, sl]),
                "wk": warr(np.ascontiguousarray(wk[:, sl])),
                "wv": warr(wv[:, sl]),
                "bq": np.ascontiguousarray(bq[sl].reshape(128, 1)),
                "bk": np.ascontiguousarray(bk[sl].reshape(128, 1)),
                "bv": np.ascontiguousarray(bv[sl].reshape(128, 1)),
                "wp": wp_full,
                "bp": bp,
            }
        )
    return in_maps


# token shard: core c's 256 output columns per batch are tokens
# [c*128, (c+1)*128) from the first q-half and 1024 + the same from the
# second q-half (see finish staging / fire_a2a in build_nc)
_TOK_IDX = np.concatenate(
    [np.r_[c * 128 : (c + 1) * 128, 1024 + c * 128 : 1024 + (c + 1) * 128]
     for c in range(NCORES)]
)


def run_sharded(inputs, trace=False, **kw):
    nc = _get_nc()
    in_maps = _make_in_maps(inputs)
    res = run_bass_kernel_spmd(nc, in_maps, list(range(NCORES)), trace=trace, **kw)
    yT = np.concatenate([res.results[c]["out"] for c in range(NCORES)], axis=2)
    out = np.empty((B, N, C), dtype=np.float32)
    out[:, _TOK_IDX, :] = yT.transpose(0, 2, 1)
    return out, res


def kernel(**inputs) -> np.ndarray:
    out, _ = run_sharded(inputs, trace=False)
    return out
